# revision 23
# baseline (speedup 1.0000x reference)
# Multi-head causal attention (B=4, S=2048, D=1024, H=16) on 8 TRN2 NeuronCores.
#
# Sharding: batch x query-chunk. Core c handles batch b=c//2 and two 512-row
# query chunks of that batch: cores with c%2==0 take real chunks (0, 3),
# c%2==1 take (1, 2). The SPMD program is identical on every core: it
# processes two query "slots" with fixed kk-tile capacities (8, 16); real
# chunk needs (4,8,12,16 tiles) are mapped into those capacities and the
# excess key tiles are zeroed by per-core causal-mask input data. Each core
# computes K/V projections for its whole batch (duplicated across the 2 cores
# sharing a batch) so no cross-core collectives are needed.
#
# Matmuls run in bf16 (fp32 PSUM accumulation); softmax statistics stay fp32.
# K/Q/V/OT all live in SBUF for the whole kernel -- no DRAM spills.
# Attention uses the transposed-scores layout St[kk, q]:
#   Kt[d, s], Qt[d, q]; St = Kt_tile.T @ Qt  (2 heads packed into one 2-bank
#   PSUM tile, exp'd in a single ACT op)
#   P = exp(St) * mask
#   OT[dv, q] += V_aug[kk, 65].T @ P   -- V carries a ones column, so PSUM
#     row 64 accumulates the softmax denominators for free.
#   OT_norm = OT * reciprocal(bcast(denoms)); y = sum_dc OT.T @ woT + b_o.
import sys

if '/opt/trn_rl_repo' not in sys.path:
    sys.path.insert(0, '/opt/trn_rl_repo')

import numpy as np

B, S, D = 4, 2048, 1024
H, DK = 16, 64
NCORES = 8
SC = 512
NKT = S // 128            # 16 kk tiles
HPN = D // 128            # 8 head-pairs
CAPS = (8, 16)            # kk-tile capacity per slot (uniform across cores)
CHUNKS = [(0, 3), (1, 2)]  # real chunk pair per core parity

_CACHE = {}


def _build_program():
    import contextlib

    import concourse.tile as tile
    from concourse import bacc, mybir

    F32 = mybir.dt.float32
    BF16 = mybir.dt.bfloat16
    EXP = mybir.ActivationFunctionType.Exp

    nc = bacc.Bacc("TRN2", target_bir_lowering=False, debug=False,
                   num_devices=NCORES)

    xT_d = nc.dram_tensor("xT", [D, S], BF16, kind="ExternalInput")
    xQT_d = nc.dram_tensor("xQT", [D, 2 * SC], BF16, kind="ExternalInput")
    wqT_d = nc.dram_tensor("wqT", [D, D], BF16, kind="ExternalInput")
    wkT_d = nc.dram_tensor("wkT", [D, D], BF16, kind="ExternalInput")
    wvT_d = nc.dram_tensor("wvT", [D, D], BF16, kind="ExternalInput")
    woT_d = nc.dram_tensor("woT", [D, D], BF16, kind="ExternalInput")
    bias_d = nc.dram_tensor("bias", [1, D], BF16, kind="ExternalInput")
    masks_d = nc.dram_tensor("masks", [128, NKT * 1024], BF16,
                             kind="ExternalInput")
    y_d = nc.dram_tensor("y", [2 * SC, D], F32, kind="ExternalOutput")

    with tile.TileContext(nc) as tc, contextlib.ExitStack() as ctx:
        smalls = ctx.enter_context(tc.tile_pool(name="smalls", bufs=1))
        p_OT = ctx.enter_context(tc.tile_pool(name="otp", bufs=1))
        p_Kt = ctx.enter_context(tc.tile_pool(name="ktp", bufs=1))
        p_Qt = ctx.enter_context(tc.tile_pool(name="qtp", bufs=1))
        p_V = ctx.enter_context(tc.tile_pool(name="vp", bufs=1))
        p_mk = ctx.enter_context(tc.tile_pool(name="mk", bufs=1))

        masks_sb = p_mk.tile([128, NKT * 1024], BF16, tag="masks")
        nc.gpsimd.dma_start(masks_sb[:], masks_d.ap())

        bias_sb = smalls.tile([1, D], BF16, tag="bias")
        nc.sync.dma_start(bias_sb[:], bias_d.ap())
        ones1f = smalls.tile([1, 128], F32, tag="ones1f")
        nc.vector.memset(ones1f[:], 1.0)
        ones1 = smalls.tile([1, 128], BF16, tag="ones1")
        nc.vector.tensor_copy(ones1[:], ones1f[:])
        ones256f = smalls.tile([128, 256], F32, tag="ones256f")
        nc.vector.memset(ones256f[:], 1.0)

        OT = p_OT.tile([128, HPN * 2 * SC], BF16, tag="OT")
        Kt = p_Kt.tile([128, HPN * S], BF16, tag="Kt")
        Qt = p_Qt.tile([128, HPN * 2 * SC], BF16, tag="Qt")
        Vsb = p_V.tile([128, NKT * H * 65], BF16, tag="Vsb")

        # ones columns of V_aug (all 16 s-tiles, one strided copy)
        nc.vector.tensor_copy(
            Vsb[:].rearrange("p (s h c) -> p s h c", s=NKT, c=65)
            [:, :, :, 64:65],
            ones256f[:].rearrange("p (s h) -> p s h", s=NKT)[:, :, :, None])

        # ---- V + K projections, one half of the sequence at a time ----
        with tc.tile_pool(name="xth", bufs=2) as p_xh, \
             tc.tile_pool(name="wfv", bufs=1) as p_wv, \
             tc.tile_pool(name="wfk", bufs=1) as p_wk, \
             tc.tile_pool(name="psp", bufs=8, space="PSUM") as psp:
            wv = p_wv.tile([128, 8 * D], BF16, tag="wv")
            wk = p_wk.tile([128, 8 * D], BF16, tag="wk")
            xhs = [p_xh.tile([128, 8 * 1024], BF16, tag="xh",
                             name=f"xh_{h}") for h in range(2)]
            # interleave so the first V matmul group's inputs land first
            for k in range(8):
                nc.sync.dma_start(
                    xhs[0][:, k * 1024:(k + 1) * 1024],
                    xT_d.ap()[k * 128:(k + 1) * 128, 0:1024])
                nc.sync.dma_start(
                    wv[:, k * D:(k + 1) * D],
                    wvT_d.ap()[k * 128:(k + 1) * 128, :])
            for k in range(8):
                nc.sync.dma_start(
                    wk[:, k * D:(k + 1) * D],
                    wkT_d.ap()[k * 128:(k + 1) * 128, :])
                nc.sync.dma_start(
                    xhs[1][:, k * 1024:(k + 1) * 1024],
                    xT_d.ap()[k * 128:(k + 1) * 128, 1024:2048])

            for half in range(2):
                xh = xhs[half]
                # V for the 8 s-tiles of this half (into SBUF V_aug layout)
                for sti in range(8):
                    st_g = half * 8 + sti
                    for dvc in range(2):
                        ps = psp.tile([128, 512], F32, tag="ps")
                        for k in range(8):
                            nc.tensor.matmul(
                                ps[:],
                                xh[:, k * 1024 + sti * 128:
                                   k * 1024 + (sti + 1) * 128],
                                wv[:, k * D + dvc * 512:k * D + (dvc + 1) * 512],
                                start=(k == 0), stop=(k == 7))
                        off = st_g * 1040 + dvc * 520
                        nc.vector.tensor_copy(
                            Vsb[:, off:off + 520]
                            .rearrange("p (h c) -> p h c", c=65)[:, :, 0:64],
                            ps[:].rearrange("p (h c) -> p h c", c=64))
                # K for the 2 s-chunks of this half -> SBUF-resident Kt
                for sc2 in range(2):
                    sc = half * 2 + sc2
                    ps8 = [psp.tile([128, 512], F32, tag="ps",
                                    name=f"psk_{sc}_{hp}")
                           for hp in range(HPN)]
                    for k in range(8):
                        for hp in range(HPN):
                            nc.tensor.matmul(
                                ps8[hp][:],
                                wk[:, k * D + hp * 128:k * D + (hp + 1) * 128],
                                xh[:, k * 1024 + sc2 * 512:
                                   k * 1024 + (sc2 + 1) * 512],
                                start=(k == 0), stop=(k == 7))
                    for hp in range(HPN):
                        nc.vector.tensor_copy(
                            Kt[:, hp * S + sc * 512:hp * S + (sc + 1) * 512],
                            ps8[hp][:])

        # ------------- Q projection (xQT streamed, wq resident) ----------
        with tc.tile_pool(name="wf2", bufs=1) as p_w2, \
             tc.tile_pool(name="xqs", bufs=4) as p_xq, \
             tc.tile_pool(name="psq", bufs=8, space="PSUM") as psq:
            wq = p_w2.tile([128, 8 * D], BF16, tag="w2")
            for k in range(8):
                nc.sync.dma_start(
                    wq[:, k * D:(k + 1) * D],
                    wqT_d.ap()[k * 128:(k + 1) * 128, :])
            for ci in range(2):
                ps8 = [psq.tile([128, 512], F32, tag="ps",
                                name=f"psq_{ci}_{hp}") for hp in range(HPN)]
                for k in range(8):
                    xq1 = p_xq.tile([128, 512], BF16, tag="xq")
                    nc.sync.dma_start(
                        xq1[:],
                        xQT_d.ap()[k * 128:(k + 1) * 128,
                                   ci * SC:(ci + 1) * SC])
                    for hp in range(HPN):
                        nc.tensor.matmul(
                            ps8[hp][:],
                            wq[:, k * D + hp * 128:k * D + (hp + 1) * 128],
                            xq1[:], start=(k == 0), stop=(k == 7))
                for hp in range(HPN):
                    nc.vector.tensor_copy(
                        Qt[:, hp * 2 * SC + ci * SC:
                           hp * 2 * SC + (ci + 1) * SC],
                        ps8[hp][:])

        # ------------- attention + interleaved output projection ---------
        with tc.tile_pool(name="rs", bufs=2) as p_rs, \
             tc.tile_pool(name="bcp", bufs=1) as p_bc, \
             tc.tile_pool(name="pp", bufs=8) as p_P, \
             tc.tile_pool(name="wo", bufs=1) as p_wo, \
             tc.tile_pool(name="ybp", bufs=4) as p_yb, \
             tc.tile_pool(name="pst", bufs=2, space="PSUM") as p_st, \
             tc.tile_pool(name="pav", bufs=4, space="PSUM") as p_av:

            wo = p_wo.tile([128, 8 * D], BF16, tag="wo")
            for k in range(8):
                nc.sync.dma_start(
                    wo[:, k * D:(k + 1) * D],
                    woT_d.ap()[k * 128:(k + 1) * 128, :])

            for ci, cap in enumerate(CAPS):
                for bl in range(HPN // 2):
                    av = [p_av.tile([128, 512], F32, tag="av",
                                    name=f"av_{ci}_{bl}_{i}")
                          for i in range(4)]

                    def emit_av(t, p_tiles, cap=cap, av=av, bl=bl):
                        for hp_i in range(2):
                            for hh in range(2):
                                hi = 2 * hp_i + hh
                                off = (t * 1040 + (2 * bl + hp_i) * 130 +
                                       hh * 65)
                                nc.tensor.matmul(
                                    av[hi][0:65, :],
                                    Vsb[:, off:off + 65],
                                    p_tiles[hp_i][:, hh * 512:(hh + 1) * 512],
                                    start=(t == 0), stop=(t == cap - 1))

                    pending = []
                    for t in range(cap):
                        p_cur = []
                        for hp_i in range(2):
                            hp = 2 * bl + hp_i
                            st = p_st.tile([128, 1024], F32, tag="st")
                            for hh in range(2):
                                r0 = 64 * hh
                                nc.tensor.matmul(
                                    st[:, hh * 512:(hh + 1) * 512],
                                    Kt[r0:r0 + 64,
                                       hp * S + t * 128:hp * S + (t + 1) * 128],
                                    Qt[r0:r0 + 64,
                                       hp * 2 * SC + ci * SC:
                                       hp * 2 * SC + (ci + 1) * SC],
                                    start=True, stop=True,
                                    tile_position=(r0, 0))
                            p1 = p_P.tile([128, 1024], BF16, tag="p")
                            nc.scalar.activation(p1[:], st[:], EXP)
                            if ci == 0 or t >= 8:
                                p2 = p_P.tile([128, 1024], BF16, tag="p")
                                nc.vector.tensor_mul(
                                    p2[:], p1[:],
                                    masks_sb[:, t * 1024:(t + 1) * 1024])
                                p1 = p2
                            p_cur.append(p1)
                        # lag-2 software pipeline: exp(t) overlaps the PE
                        # work of scores(t..t+1) + AV(t-2..t-1)
                        pending.append((t, p_cur))
                        if len(pending) > 2:
                            tt, pp_t = pending.pop(0)
                            emit_av(tt, pp_t)
                    for tt, pp_t in pending:
                        emit_av(tt, pp_t)
                    # normalize, one head-pair at a time
                    for hp_i in range(2):
                        hp = 2 * bl + hp_i
                        rs = p_rs.tile([1, 1024], F32, tag="rs")
                        for hh in range(2):
                            hi = 2 * hp_i + hh
                            nc.vector.tensor_copy(
                                rs[0:1, hh * 512:hh * 512 + 512],
                                av[hi][64:65, :])
                        bc = p_bc.tile([128, 1024], F32, tag="bc")
                        nc.gpsimd.partition_broadcast(bc[:], rs[:])
                        rbc = p_bc.tile([128, 1024], F32, tag="rbc")
                        scr = p_bc.tile([128, 1024], F32, tag="scr")
                        nc.vector.reciprocal_approx_accurate(
                            rbc[:], bc[:], scratch=scr[:])
                        for hh in range(2):
                            hi = 2 * hp_i + hh
                            r0 = 64 * hh
                            nc.vector.tensor_mul(
                                OT[r0:r0 + 64,
                                   hp * 2 * SC + ci * SC:
                                   hp * 2 * SC + (ci + 1) * SC],
                                av[hi][0:64, :],
                                rbc[r0:r0 + 64, hh * 512:hh * 512 + 512])

                # output projection for this ci's 512 query rows -- fills
                # the PE while the next ci's attention pipeline ramps up
                for qi in range(4 * ci, 4 * ci + 4):
                    for nc2 in range(2):
                        ps = p_av.tile([128, 512], F32, tag="av",
                                       name=f"psy_{qi}_{nc2}")
                        for dc in range(8):
                            nc.tensor.matmul(
                                ps[:],
                                OT[:, dc * 2 * SC + qi * 128:
                                   dc * 2 * SC + (qi + 1) * 128],
                                wo[:, dc * D + nc2 * 512:
                                   dc * D + (nc2 + 1) * 512],
                                start=(dc == 0), stop=False)
                        nc.tensor.matmul(
                            ps[:], ones1[:],
                            bias_sb[0:1, nc2 * 512:(nc2 + 1) * 512],
                            start=False, stop=True)
                        yb = p_yb.tile([128, 512], F32, tag="yb")
                        nc.vector.tensor_copy(yb[:], ps[:])
                        nc.sync.dma_start(
                            y_d.ap()[qi * 128:(qi + 1) * 128,
                                     nc2 * 512:(nc2 + 1) * 512], yb[:])

    nc.compile()
    return nc


def _get_program():
    if 'nc' not in _CACHE:
        _CACHE['nc'] = _build_program()
    return _CACHE['nc']


def _tri_masks():
    p = np.arange(128)[:, None]
    f = np.arange(SC)[None, :]
    return [(p <= f - 128 * r).astype(np.float32) for r in range(4)]


def _masks_for_core(c):
    import ml_dtypes
    tri = _tri_masks()
    ones = np.ones((128, SC), np.float32)
    zeros = np.zeros((128, SC), np.float32)
    j_pair = CHUNKS[c % 2]
    out = np.zeros((128, NKT * 1024), np.float32)
    for ci, cap in enumerate(CAPS):
        j = j_pair[ci]
        t0 = 0 if ci == 0 else 8
        for t in range(t0, cap):
            if t < 4 * j:
                m = ones
            elif t < 4 * j + 4:
                m = tri[t - 4 * j]
            else:
                m = zeros
            out[:, t * 1024:t * 1024 + 512] = m
            out[:, t * 1024 + 512:(t + 1) * 1024] = m
    return out.astype(ml_dtypes.bfloat16)


def kernel(x, w_q, w_k, w_v, w_o, b_o):
    import ml_dtypes
    from concourse.bass_utils import run_bass_kernel_spmd

    BF = ml_dtypes.bfloat16
    x = np.asarray(x, dtype=np.float32)
    nc = _get_program()

    scale = np.float32(1.0 / np.sqrt(DK))
    common = {
        "wqT": np.ascontiguousarray(
            (np.asarray(w_q, np.float32).T * scale)).astype(BF),
        "wkT": np.ascontiguousarray(np.asarray(w_k, np.float32).T).astype(BF),
        "wvT": np.ascontiguousarray(np.asarray(w_v, np.float32).T).astype(BF),
        "woT": np.ascontiguousarray(np.asarray(w_o, np.float32).T).astype(BF),
        "bias": np.asarray(b_o, np.float32)[None, :].astype(BF),
    }

    in_maps = []
    for c in range(NCORES):
        b = c // 2
        j1, j2 = CHUNKS[c % 2]
        xb = x[b]
        xq = np.concatenate(
            [xb[j1 * SC:(j1 + 1) * SC], xb[j2 * SC:(j2 + 1) * SC]], axis=0)
        in_maps.append({
            "xT": np.ascontiguousarray(xb.T).astype(BF),
            "xQT": np.ascontiguousarray(xq.T).astype(BF),
            "masks": _masks_for_core(c),
            **common,
        })

    res = run_bass_kernel_spmd(nc, in_maps, core_ids=list(range(NCORES)),
                               trace=_CACHE.get('trace', False),
                               tmpdir=_CACHE.get('tmpdir'))
    _CACHE['last_res'] = res

    y = np.empty((B, S, D), dtype=np.float32)
    for c in range(NCORES):
        b = c // 2
        j1, j2 = CHUNKS[c % 2]
        yc = res.results[c]["y"]
        y[b, j1 * SC:(j1 + 1) * SC] = yc[0:SC]
        y[b, j2 * SC:(j2 + 1) * SC] = yc[SC:2 * SC]
    return y


# revision 24
# speedup vs baseline: 1.1877x; 1.1877x over previous
# Multi-head causal attention (B=4, S=2048, D=1024, H=16) on 8 TRN2 NeuronCores.
#
# Sharding: batch x query-chunk. Core c handles batch b=c//2 and two 512-row
# query chunks of that batch: cores with c%2==0 take real chunks (0, 3),
# c%2==1 take (1, 2). The SPMD program is identical on every core: it
# processes two query "slots" with fixed kk-tile capacities (8, 16); real
# chunk needs (4,8,12,16 tiles) are mapped into those capacities and the
# excess key tiles are zeroed by per-core causal-mask input data. Each core
# computes K/V projections for its whole batch (duplicated across the 2 cores
# sharing a batch) so no cross-core collectives are needed.
#
# Matmuls run in bf16 (fp32 PSUM accumulation); softmax statistics stay fp32.
# K/Q/V/OT all live in SBUF for the whole kernel -- no DRAM spills.
# Attention uses the transposed-scores layout St[kk, q]:
#   Kt[d, s], Qt[d, q]; St = Kt_tile.T @ Qt  (2 heads packed into one 2-bank
#   PSUM tile, exp'd in a single ACT op)
#   P = exp(St) * mask
#   OT[dv, q] += V_aug[kk, 65].T @ P   -- V carries a ones column, so PSUM
#     row 64 accumulates the softmax denominators for free.
#   OT_norm = OT * reciprocal(bcast(denoms)); y = sum_dc OT.T @ woT + b_o.
import sys

if '/opt/trn_rl_repo' not in sys.path:
    sys.path.insert(0, '/opt/trn_rl_repo')

import numpy as np

B, S, D = 4, 2048, 1024
H, DK = 16, 64
NCORES = 8
SC = 512
NKT = S // 128            # 16 kk tiles
HPN = D // 128            # 8 head-pairs
CAPS = (8, 16)            # kk-tile capacity per slot (uniform across cores)
CHUNKS = [(0, 3), (1, 2)]  # real chunk pair per core parity

_CACHE = {}


def _build_program():
    import contextlib

    import concourse.tile as tile
    from concourse import bacc, mybir

    F32 = mybir.dt.float32
    BF16 = mybir.dt.bfloat16
    EXP = mybir.ActivationFunctionType.Exp

    nc = bacc.Bacc("TRN2", target_bir_lowering=False, debug=False,
                   num_devices=NCORES)

    xT_d = nc.dram_tensor("xT", [D, S], BF16, kind="ExternalInput")
    xQT_d = nc.dram_tensor("xQT", [D, 2 * SC], BF16, kind="ExternalInput")
    wqT_d = nc.dram_tensor("wqT", [D, D], BF16, kind="ExternalInput")
    wkT_d = nc.dram_tensor("wkT", [D, D], BF16, kind="ExternalInput")
    wvT_d = nc.dram_tensor("wvT", [D, D], BF16, kind="ExternalInput")
    woT_d = nc.dram_tensor("woT", [D, D], BF16, kind="ExternalInput")
    bias_d = nc.dram_tensor("bias", [1, D], BF16, kind="ExternalInput")
    masks_d = nc.dram_tensor("masks", [128, NKT * 1024], BF16,
                             kind="ExternalInput")
    y_d = nc.dram_tensor("y", [2 * SC, D], F32, kind="ExternalOutput")

    with tile.TileContext(nc) as tc, contextlib.ExitStack() as ctx:
        smalls = ctx.enter_context(tc.tile_pool(name="smalls", bufs=1))
        p_OT = ctx.enter_context(tc.tile_pool(name="otp", bufs=1))
        p_Kt = ctx.enter_context(tc.tile_pool(name="ktp", bufs=1))
        p_Qt = ctx.enter_context(tc.tile_pool(name="qtp", bufs=1))
        p_V = ctx.enter_context(tc.tile_pool(name="vp", bufs=1))
        p_mk = ctx.enter_context(tc.tile_pool(name="mk", bufs=1))

        masks_sb = p_mk.tile([128, NKT * 1024], BF16, tag="masks")
        nc.gpsimd.dma_start(masks_sb[:], masks_d.ap())

        bias_sb = smalls.tile([1, D], BF16, tag="bias")
        nc.sync.dma_start(bias_sb[:], bias_d.ap())
        ones1f = smalls.tile([1, 128], F32, tag="ones1f")
        nc.vector.memset(ones1f[:], 1.0)
        ones1 = smalls.tile([1, 128], BF16, tag="ones1")
        nc.vector.tensor_copy(ones1[:], ones1f[:])
        ones256f = smalls.tile([128, 256], F32, tag="ones256f")
        nc.vector.memset(ones256f[:], 1.0)

        OT = p_OT.tile([128, HPN * 2 * SC], BF16, tag="OT")
        Kt = p_Kt.tile([128, HPN * S], BF16, tag="Kt")
        Qt = p_Qt.tile([128, HPN * 2 * SC], BF16, tag="Qt")
        Vsb = p_V.tile([128, NKT * H * 65], BF16, tag="Vsb")

        # ones columns of V_aug (all 16 s-tiles, one strided copy)
        nc.vector.tensor_copy(
            Vsb[:].rearrange("p (s h c) -> p s h c", s=NKT, c=65)
            [:, :, :, 64:65],
            ones256f[:].rearrange("p (s h) -> p s h", s=NKT)[:, :, :, None])

        # ---- V + K projections, one half of the sequence at a time ----
        with tc.tile_pool(name="xth", bufs=2) as p_xh, \
             tc.tile_pool(name="wfv", bufs=1) as p_wv, \
             tc.tile_pool(name="wfk", bufs=1) as p_wk, \
             tc.tile_pool(name="psp", bufs=8, space="PSUM") as psp:
            wv = p_wv.tile([128, 8 * D], BF16, tag="wv")
            wk = p_wk.tile([128, 8 * D], BF16, tag="wk")
            xhs = [p_xh.tile([128, 8 * 1024], BF16, tag="xh",
                             name=f"xh_{h}") for h in range(2)]
            # interleave so the first V matmul group's inputs land first
            for k in range(8):
                nc.sync.dma_start(
                    xhs[0][:, k * 1024:(k + 1) * 1024],
                    xT_d.ap()[k * 128:(k + 1) * 128, 0:1024])
                nc.sync.dma_start(
                    wv[:, k * D:(k + 1) * D],
                    wvT_d.ap()[k * 128:(k + 1) * 128, :])
            for k in range(8):
                nc.sync.dma_start(
                    wk[:, k * D:(k + 1) * D],
                    wkT_d.ap()[k * 128:(k + 1) * 128, :])
                nc.sync.dma_start(
                    xhs[1][:, k * 1024:(k + 1) * 1024],
                    xT_d.ap()[k * 128:(k + 1) * 128, 1024:2048])

            for half in range(2):
                xh = xhs[half]
                # V for the 8 s-tiles of this half (into SBUF V_aug layout)
                for sti in range(8):
                    st_g = half * 8 + sti
                    for dvc in range(2):
                        ps = psp.tile([128, 512], F32, tag="ps")
                        for k in range(8):
                            nc.tensor.matmul(
                                ps[:],
                                xh[:, k * 1024 + sti * 128:
                                   k * 1024 + (sti + 1) * 128],
                                wv[:, k * D + dvc * 512:k * D + (dvc + 1) * 512],
                                start=(k == 0), stop=(k == 7))
                        off = st_g * 1040 + dvc * 520
                        nc.vector.tensor_copy(
                            Vsb[:, off:off + 520]
                            .rearrange("p (h c) -> p h c", c=65)[:, :, 0:64],
                            ps[:].rearrange("p (h c) -> p h c", c=64))
                # K for the 2 s-chunks of this half -> SBUF-resident Kt
                for sc2 in range(2):
                    sc = half * 2 + sc2
                    ps8 = [psp.tile([128, 512], F32, tag="ps",
                                    name=f"psk_{sc}_{hp}")
                           for hp in range(HPN)]
                    for k in range(8):
                        for hp in range(HPN):
                            nc.tensor.matmul(
                                ps8[hp][:],
                                wk[:, k * D + hp * 128:k * D + (hp + 1) * 128],
                                xh[:, k * 1024 + sc2 * 512:
                                   k * 1024 + (sc2 + 1) * 512],
                                start=(k == 0), stop=(k == 7))
                    for hp in range(HPN):
                        nc.vector.tensor_copy(
                            Kt[:, hp * S + sc * 512:hp * S + (sc + 1) * 512],
                            ps8[hp][:])

        # ------------- Q projection (xQT streamed, wq resident) ----------
        with tc.tile_pool(name="wf2", bufs=1) as p_w2, \
             tc.tile_pool(name="xqs", bufs=4) as p_xq, \
             tc.tile_pool(name="psq", bufs=8, space="PSUM") as psq:
            wq = p_w2.tile([128, 8 * D], BF16, tag="w2")
            for k in range(8):
                nc.sync.dma_start(
                    wq[:, k * D:(k + 1) * D],
                    wqT_d.ap()[k * 128:(k + 1) * 128, :])
            for ci in range(2):
                ps8 = [psq.tile([128, 512], F32, tag="ps",
                                name=f"psq_{ci}_{hp}") for hp in range(HPN)]
                for k in range(8):
                    xq1 = p_xq.tile([128, 512], BF16, tag="xq")
                    nc.sync.dma_start(
                        xq1[:],
                        xQT_d.ap()[k * 128:(k + 1) * 128,
                                   ci * SC:(ci + 1) * SC])
                    for hp in range(HPN):
                        nc.tensor.matmul(
                            ps8[hp][:],
                            wq[:, k * D + hp * 128:k * D + (hp + 1) * 128],
                            xq1[:], start=(k == 0), stop=(k == 7))
                for hp in range(HPN):
                    nc.vector.tensor_copy(
                        Qt[:, hp * 2 * SC + ci * SC:
                           hp * 2 * SC + (ci + 1) * SC],
                        ps8[hp][:])

        # ------------- attention + interleaved output projection ---------
        with tc.tile_pool(name="rs", bufs=2) as p_rs, \
             tc.tile_pool(name="bcp", bufs=1) as p_bc, \
             tc.tile_pool(name="pp", bufs=8) as p_P, \
             tc.tile_pool(name="wo", bufs=1) as p_wo, \
             tc.tile_pool(name="ybp", bufs=4) as p_yb, \
             tc.tile_pool(name="pst", bufs=2, space="PSUM") as p_st, \
             tc.tile_pool(name="pav", bufs=4, space="PSUM") as p_av:

            wo = p_wo.tile([128, 8 * D], BF16, tag="wo")
            for k in range(8):
                nc.sync.dma_start(
                    wo[:, k * D:(k + 1) * D],
                    woT_d.ap()[k * 128:(k + 1) * 128, :])

            for ci, cap in enumerate(CAPS):
                for bl in range(HPN // 2):
                    av = [p_av.tile([128, 512], F32, tag="av",
                                    name=f"av_{ci}_{bl}_{i}")
                          for i in range(4)]

                    def emit_av(t, p_tiles, cap=cap, av=av, bl=bl):
                        for hp_i in range(2):
                            for hh in range(2):
                                hi = 2 * hp_i + hh
                                off = (t * 1040 + (2 * bl + hp_i) * 130 +
                                       hh * 65)
                                nc.tensor.matmul(
                                    av[hi][0:65, :],
                                    Vsb[:, off:off + 65],
                                    p_tiles[hp_i][:, hh * 512:(hh + 1) * 512],
                                    start=(t == 0), stop=(t == cap - 1))

                    pending = []
                    for t in range(cap):
                        p_cur = []
                        for hp_i in range(2):
                            hp = 2 * bl + hp_i
                            st = p_st.tile([128, 1024], F32, tag="st")
                            for hh in range(2):
                                r0 = 64 * hh
                                nc.tensor.matmul(
                                    st[:, hh * 512:(hh + 1) * 512],
                                    Kt[r0:r0 + 64,
                                       hp * S + t * 128:hp * S + (t + 1) * 128],
                                    Qt[r0:r0 + 64,
                                       hp * 2 * SC + ci * SC:
                                       hp * 2 * SC + (ci + 1) * SC],
                                    start=True, stop=True,
                                    tile_position=(r0, 0))
                            p1 = p_P.tile([128, 1024], BF16, tag="p")
                            nc.scalar.activation(p1[:], st[:], EXP)
                            if ci == 0 or t >= 8:
                                p2 = p_P.tile([128, 1024], BF16, tag="p")
                                nc.vector.tensor_mul(
                                    p2[:], p1[:],
                                    masks_sb[:, t * 1024:(t + 1) * 1024])
                                p1 = p2
                            p_cur.append(p1)
                        # lag-2 software pipeline: exp(t) overlaps the PE
                        # work of scores(t..t+1) + AV(t-2..t-1)
                        pending.append((t, p_cur))
                        if len(pending) > 2:
                            tt, pp_t = pending.pop(0)
                            emit_av(tt, pp_t)
                    for tt, pp_t in pending:
                        emit_av(tt, pp_t)
                    # normalize, one head-pair at a time
                    for hp_i in range(2):
                        hp = 2 * bl + hp_i
                        rs = p_rs.tile([1, 1024], F32, tag="rs")
                        for hh in range(2):
                            hi = 2 * hp_i + hh
                            nc.vector.tensor_copy(
                                rs[0:1, hh * 512:hh * 512 + 512],
                                av[hi][64:65, :])
                        bc = p_bc.tile([128, 1024], F32, tag="bc")
                        nc.gpsimd.partition_broadcast(bc[:], rs[:])
                        rbc = p_bc.tile([128, 1024], F32, tag="rbc")
                        scr = p_bc.tile([128, 1024], F32, tag="scr")
                        nc.vector.reciprocal_approx_accurate(
                            rbc[:], bc[:], scratch=scr[:])
                        for hh in range(2):
                            hi = 2 * hp_i + hh
                            r0 = 64 * hh
                            nc.vector.tensor_mul(
                                OT[r0:r0 + 64,
                                   hp * 2 * SC + ci * SC:
                                   hp * 2 * SC + (ci + 1) * SC],
                                av[hi][0:64, :],
                                rbc[r0:r0 + 64, hh * 512:hh * 512 + 512])

            # ---------------- output projection ----------------
            for qi in range(8):
                for nc2 in range(2):
                    ps = p_av.tile([128, 512], F32, tag="av",
                                   name=f"psy_{qi}_{nc2}")
                    for dc in range(8):
                        nc.tensor.matmul(
                            ps[:],
                            OT[:, dc * 2 * SC + qi * 128:
                               dc * 2 * SC + (qi + 1) * 128],
                            wo[:, dc * D + nc2 * 512:
                               dc * D + (nc2 + 1) * 512],
                            start=(dc == 0), stop=False)
                    nc.tensor.matmul(
                        ps[:], ones1[:],
                        bias_sb[0:1, nc2 * 512:(nc2 + 1) * 512],
                        start=False, stop=True)
                    yb = p_yb.tile([128, 512], F32, tag="yb")
                    nc.vector.tensor_copy(yb[:], ps[:])
                    nc.sync.dma_start(
                        y_d.ap()[qi * 128:(qi + 1) * 128,
                                 nc2 * 512:(nc2 + 1) * 512], yb[:])

    nc.compile()
    return nc


def _get_program():
    if 'nc' not in _CACHE:
        _CACHE['nc'] = _build_program()
    return _CACHE['nc']


def _tri_masks():
    p = np.arange(128)[:, None]
    f = np.arange(SC)[None, :]
    return [(p <= f - 128 * r).astype(np.float32) for r in range(4)]


def _masks_for_core(c):
    import ml_dtypes
    tri = _tri_masks()
    ones = np.ones((128, SC), np.float32)
    zeros = np.zeros((128, SC), np.float32)
    j_pair = CHUNKS[c % 2]
    out = np.zeros((128, NKT * 1024), np.float32)
    for ci, cap in enumerate(CAPS):
        j = j_pair[ci]
        t0 = 0 if ci == 0 else 8
        for t in range(t0, cap):
            if t < 4 * j:
                m = ones
            elif t < 4 * j + 4:
                m = tri[t - 4 * j]
            else:
                m = zeros
            out[:, t * 1024:t * 1024 + 512] = m
            out[:, t * 1024 + 512:(t + 1) * 1024] = m
    return out.astype(ml_dtypes.bfloat16)


def kernel(x, w_q, w_k, w_v, w_o, b_o):
    import ml_dtypes
    from concourse.bass_utils import run_bass_kernel_spmd

    BF = ml_dtypes.bfloat16
    x = np.asarray(x, dtype=np.float32)
    nc = _get_program()

    scale = np.float32(1.0 / np.sqrt(DK))
    common = {
        "wqT": np.ascontiguousarray(
            (np.asarray(w_q, np.float32).T * scale)).astype(BF),
        "wkT": np.ascontiguousarray(np.asarray(w_k, np.float32).T).astype(BF),
        "wvT": np.ascontiguousarray(np.asarray(w_v, np.float32).T).astype(BF),
        "woT": np.ascontiguousarray(np.asarray(w_o, np.float32).T).astype(BF),
        "bias": np.asarray(b_o, np.float32)[None, :].astype(BF),
    }

    in_maps = []
    for c in range(NCORES):
        b = c // 2
        j1, j2 = CHUNKS[c % 2]
        xb = x[b]
        xq = np.concatenate(
            [xb[j1 * SC:(j1 + 1) * SC], xb[j2 * SC:(j2 + 1) * SC]], axis=0)
        in_maps.append({
            "xT": np.ascontiguousarray(xb.T).astype(BF),
            "xQT": np.ascontiguousarray(xq.T).astype(BF),
            "masks": _masks_for_core(c),
            **common,
        })

    res = run_bass_kernel_spmd(nc, in_maps, core_ids=list(range(NCORES)),
                               trace=_CACHE.get('trace', False),
                               tmpdir=_CACHE.get('tmpdir'))
    _CACHE['last_res'] = res

    y = np.empty((B, S, D), dtype=np.float32)
    for c in range(NCORES):
        b = c // 2
        j1, j2 = CHUNKS[c % 2]
        yc = res.results[c]["y"]
        y[b, j1 * SC:(j1 + 1) * SC] = yc[0:SC]
        y[b, j2 * SC:(j2 + 1) * SC] = yc[SC:2 * SC]
    return y
